# revision 13
# baseline (speedup 1.0000x reference)
"""Trainium2 Bass kernel for nn_Couple_loss_62380105007762.

Loss = w0 * MSE + w1 * KLD + w2 * CE where
  sig(x)  = 2 * x[:, 0].sum(axis=F)                      (inverse SSQ-STFT, real channel only)
  MSE     = sum((sig(output_rec) - sig(target_rec))**2)
  KLD     = -0.5 * sum(1 + log_var - mean**2 - exp(log_var))
  CE      = mean cross-entropy(output_clas, target_clas)

Sharding: data-parallel over the batch dim (64 rows -> 8 cores x 8 rows).
Each core emits per-shard partial sums [8, 6] (4 sq quarters, kld, ce rows);
host psums the shards and applies the 3 loss weights.

v6 (v1 72.0us, v2 38.6us, v3 31.7us, v5 28.9us):
  - fp8e4 rec data (4x traffic; ~9e-4 rel err), interleaved [F, (b, {o,t}, T)].
  - DMA under an f32 VIEW of the fp8 tiles in 2-batch-row 1 MB chunks:
    HWDGE queue rate scales with descriptor size (descriptors split at 2048
    elements; 8 KB f32 descriptors measured 215 GB/s/queue vs ~160 for fp8).
  - DMA triggers + constant memsets hoisted into `main` right after the
    runtime-preamble InstCall: they depend on nothing in-run, and runs are
    serialized by the runtime, so they fire ~1.4us before the framework
    prologue barrier completes.
  - DoubleRow fp8 matmuls (32): one matmul = sum_f(o) - sum_f(t) per out
    column via the o|t interleave (contraction over 2x128 virtual rows).
  - psum split into FOUR [8, 1024] group tiles (2 banks each): group =
    (batch half, T half). Tile tracks dependencies per tile, so each
    group's square+accumulate fires as soon as ITS last matmul stops --
    only the final [8, 1024] square (~1.0us) sits on the tail instead of
    a [8, 2048] one (~2.0us).
  - PE warmup matmuls bridge the PE prologue -> first-chunk gap so HAM
    is at K=8/8 (2.4 GHz) when data lands.
  - smalls (mean/logvar/clas) ride ONE packed [8, 522] f32 DMA on the
    otherwise-idle gpsimd SWDGE ring.
"""

import numpy as np
import ml_dtypes
from contextlib import ExitStack

import concourse.bass as bass
import concourse.tile as tile
from concourse import mybir
from concourse.bass_utils import run_bass_kernel_spmd

N_CORES = 8
B, Z, F, T, C = 64, 256, 128, 2048, 5
BS = B // N_CORES   # batch rows per core
HB = BS // 2        # rows per batch half
WCOL = BS * 2 * T   # interleaved free dim: 32768 fp8 columns
WCOL32 = WCOL // 4  # same bytes as f32 columns
N_CHUNK = 512       # matmul output free dim (PSUM bank limit in fp32)
KQ = T // N_CHUNK   # 4 output slices per b
N_WARM = 10         # dummy matmuls bridging PE prologue -> first 1MB chunk

FP8 = mybir.dt.float8e4
NP_FP8 = ml_dtypes.float8_e4m3
FP32 = mybir.dt.float32
AX = mybir.AxisListType
ALU = mybir.AluOpType
ACTF = mybir.ActivationFunctionType
DR = mybir.MatmulPerfMode.DoubleRow
ET = mybir.EngineType

# packed smalls layout: [BS, SM_W] f32
SM_MEAN = 0               # cols [0, 256)    mean
SM_LV = Z                 # cols [256, 512)  log_var
SM_OC = 2 * Z             # cols [512, 517)  output_clas
SM_OH = 2 * Z + C         # cols [517, 522)  one-hot(target_clas)
SM_W = 2 * Z + 2 * C

# out columns: [sq_q0..sq_q3, kld, ce]
NO = 6


def build_bass(legalize: bool = True):
    nc = bass.Bass()

    ot_rec = nc.declare_dram_parameter("ot_rec", [F, WCOL32], FP32, isOutput=False)
    smalls = nc.declare_dram_parameter("smalls", [BS, SM_W], FP32, isOutput=False)
    out = nc.declare_dram_parameter("out", [BS, NO], FP32, isOutput=True)

    hoist_hwdge = []   # instruction names to move right after the main InstCall
    hoist_pool = []    # ... and after the Pool preamble memsets

    with tile.TileContext(nc) as tc:
        with ExitStack() as ctx:
            const_pool = ctx.enter_context(tc.tile_pool(name="const", bufs=1))
            d_pool = ctx.enter_context(tc.tile_pool(name="dpool", bufs=BS // 2))
            ps_pool = ctx.enter_context(tc.tile_pool(name="ps", bufs=1, space="PSUM"))
            small = ctx.enter_context(tc.tile_pool(name="small", bufs=1))

            # ---- big data chunks; DMA issued under an f32 view ----
            QUEUES = {0: nc.sync, 1: nc.scalar, 2: nc.sync, 3: nc.scalar}
            sm_t = small.tile([BS, SM_W], FP32, tag="sm")
            i_sm = nc.gpsimd.dma_start(sm_t[:], smalls[:, :])
            hoist_pool.append(i_sm.ins.name)
            # 2-b chunks: [128, 8192] fp8 = 1 MB, 8 KB f32-view descriptors
            pairs = []
            for p in range(BS // 2):
                ch = d_pool.tile([F, 4 * T], FP8, tag="d")
                sl32 = slice(p * T, (p + 1) * T)
                i_d = QUEUES[p].dma_start(ch[:].bitcast(FP32), ot_rec[:, sl32])
                hoist_hwdge.append(i_d.ins.name)
                pairs.append(ch)
            chunks = []
            for b in range(BS):
                off = (b % 2) * 2 * T
                chunks.append(pairs[b // 2][:, off:off + 2 * T])

            # ---- constants (no DMA): selector weights + warmup junk ----
            # W[:, 8] = +1, W[:, 24] = -1, rest 0.  DoubleRow stationary for
            # batch row b: W viewed as [128, j:2(x16), m:8] at offset 8-b
            # => (j=0, m=b) hits col 8 (+1), (j=1, m=b) hits col 24 (-1).
            w_sel = const_pool.tile([F, 32], FP8, tag="wsel")
            i1 = nc.vector.memset(w_sel[:], 0.0)
            i2 = nc.vector.memset(w_sel[:, 8:9], 1.0)
            i3 = nc.vector.memset(w_sel[:, 24:25], -1.0)
            warm_in = const_pool.tile([F, N_CHUNK], FP8, tag="warmin")
            i4 = nc.vector.memset(warm_in[:], 0.0)
            hoist_hwdge += [i1.ins.name, i2.ins.name, i3.ins.name, i4.ins.name]

            # psum: four [8, 1024] group tiles (2 banks each) = all 8 banks.
            # group q = (batch half, T half); per-group squares can then
            # fire on their own group's last matmul (Tile deps are
            # tile-granular).
            ps_q0 = ps_pool.tile([BS, 2 * N_CHUNK], FP32, tag="q0")
            ps_q1 = ps_pool.tile([BS, 2 * N_CHUNK], FP32, tag="q1")
            ps_q2 = ps_pool.tile([BS, 2 * N_CHUNK], FP32, tag="q2")
            ps_q3 = ps_pool.tile([BS, 2 * N_CHUNK], FP32, tag="q3")
            ps_q = [ps_q0, ps_q1, ps_q2, ps_q3]
            # out tile: [8, 6] = sq_q0..q3 | kld | ce
            sums = small.tile([BS, NO], FP32, tag="sums")

            # ---- PE warmup: HAM unthrottles after ~3.4us of activity.
            # Writes [1, 512] garbage into q3; the first real q3 matmul
            # opens its accumulation group with start=True, clearing it.
            for i in range(N_WARM):
                nc.tensor.matmul(ps_q3[0:1, 0:N_CHUNK], w_sel[:, 0:1],
                                 warm_in[:], start=True, stop=True,
                                 skip_group_check=True)

            # ---- KLD / CE on the packed smalls tile (off critical path) ----
            m_t = sm_t[:, SM_MEAN:SM_MEAN + Z]
            lv_t = sm_t[:, SM_LV:SM_LV + Z]
            oc_t = sm_t[:, SM_OC:SM_OC + C]
            oh_t = sm_t[:, SM_OH:SM_OH + C]

            msq_sum = small.tile([BS, 1], FP32, tag="msq")
            e_sum = small.tile([BS, 1], FP32, tag="esum")
            lv_sum = small.tile([BS, 1], FP32, tag="lvsum")
            kl_junk = small.tile([BS, Z], FP32, tag="klj")
            kl_junk2 = small.tile([BS, Z], FP32, tag="klj2")
            kl_tmp = small.tile([BS, 1], FP32, tag="kltmp")
            nc.vector.tensor_tensor(kl_junk[:], m_t, m_t, ALU.mult)
            nc.vector.reduce_sum(msq_sum[:], kl_junk[:], axis=AX.X)
            nc.scalar.activation(kl_junk2[:], lv_t, ACTF.Exp, accum_out=e_sum[:])
            nc.vector.reduce_sum(lv_sum[:], lv_t, axis=AX.X)
            nc.vector.tensor_tensor(kl_tmp[:], lv_sum[:], msq_sum[:], ALU.subtract)
            nc.vector.tensor_tensor(sums[:, 4:5], kl_tmp[:], e_sum[:], ALU.subtract)

            # CE rows: ce_row = rowmax + log(sum(exp(oc - rowmax))) - oc[b, y_b]
            rowmax = small.tile([BS, 1], FP32, tag="rmax")
            nmax = small.tile([BS, 1], FP32, tag="nmax")
            sumexp = small.tile([BS, 1], FP32, tag="sexp")
            lse = small.tile([BS, 1], FP32, tag="lse")
            picked = small.tile([BS, 1], FP32, tag="picked")
            ce_junk = small.tile([BS, C], FP32, tag="cej")
            ce_junk2 = small.tile([BS, C], FP32, tag="cej2")
            ce_tmp = small.tile([BS, 1], FP32, tag="cetmp")
            nc.vector.reduce_max(rowmax[:], oc_t, axis=AX.X)
            nc.vector.tensor_scalar_mul(nmax[:], rowmax[:], -1.0)
            nc.scalar.activation(
                ce_junk[:], oc_t, ACTF.Exp, bias=nmax[:], accum_out=sumexp[:]
            )
            nc.scalar.activation(lse[:], sumexp[:], ACTF.Ln)
            nc.vector.tensor_tensor(ce_junk2[:], oc_t, oh_t, ALU.mult)
            nc.vector.reduce_sum(picked[:], ce_junk2[:], axis=AX.X)
            nc.vector.tensor_tensor(ce_tmp[:], rowmax[:], lse[:], ALU.add)
            nc.vector.tensor_tensor(sums[:, 5:6], ce_tmp[:], picked[:],
                                    ALU.subtract)

            # ---- main MSE stream ----
            # DoubleRow: out[m,n] = sum_f W3[f,0,m]*x[f,0,n] + W3[f,1,m]*x[f,1,n]
            #          = sum_f o[b, f, n] - sum_f t[b, f, n]  for row m = b
            # (b0-3 land in rows 0-3 of the q0/q1 groups; b4-7 in rows 4-7
            # of q2/q3; unused rows stay 0 and add nothing)
            w3 = w_sel[:].rearrange("p (j m) -> p j m", j=2)  # [128, 2, 16]
            for b in range(BS):
                wb = w3[:, :, 8 - b:16 - b]                    # [128, 2, 8]
                c3 = chunks[b].rearrange("p (j n) -> p j n", j=2)
                for k in range(KQ):
                    q = ps_q[(b // HB) * 2 + (k // 2)]
                    col = (k % 2) * N_CHUNK
                    nc.tensor.matmul(
                        q[:, col:col + N_CHUNK],
                        wb, c3[:, :, k * N_CHUNK:(k + 1) * N_CHUNK],
                        start=(b % HB == 0),
                        stop=(b % HB == HB - 1),
                        perf_mode=DR,
                    )
                if b % HB == HB - 1:
                    for kh in range(2):
                        qi = (b // HB) * 2 + kh
                        sq_junk = small.tile([BS, 2 * N_CHUNK], FP32,
                                             tag=f"sqj{qi}")
                        nc.scalar.activation(
                            sq_junk[:], ps_q[qi][:], ACTF.Square,
                            accum_out=sums[:, qi:qi + 1],
                        )

            nc.sync.dma_start(out[:, :], sums[:])

    if legalize:
        _legalize_multi_waits(nc)
    _hoist_preamble(nc, hoist_hwdge, hoist_pool)
    mybir.codegen_inst_isa_subclasses(nc)
    return nc


def _hoist_preamble(nc, names_after_call, names_pool):
    """Move dependency-free DMA triggers / memsets from the body block into
    `main`, ahead of the framework prologue barrier: HWDGE triggers + DVE
    memsets right after the runtime-preamble InstCall, SWDGE (Pool)
    triggers after the Pool preamble memsets (DGE ring init). Safe because
    NEFF executions are serialized by the runtime and these instructions
    depend on nothing produced in-run."""
    fn = nc.m.functions[0]
    main = fn.blocks[0]
    assert main.name == "main"
    wanted = set(names_after_call) | set(names_pool)
    moved = {}
    for blk in fn.blocks[1:]:
        keep = []
        for inst in blk.instructions:
            if inst.name in wanted:
                moved[inst.name] = inst
            else:
                keep.append(inst)
        blk.instructions = keep

    new_main = []
    for inst in main.instructions:
        new_main.append(inst)
        if type(inst).__name__ == "InstCall":
            for n in names_after_call:
                if n in moved:
                    new_main.append(moved[n])
    final = []
    pool_done = False
    for inst in new_main:
        if (not pool_done and type(inst).__name__ == "InstDrain"
                and inst.engine == ET.Pool):
            for n in names_pool:
                if n in moved:
                    final.append(moved[n])
            pool_done = True
        final.append(inst)
    main.instructions = final


def _legalize_multi_waits(nc):
    """walrus rejects TPB compute instructions carrying more than one sync
    wait. Hoist extra waits onto standalone InstEventSemaphore instructions
    on the same engine. DMA instructions keep their waits (DGE path).
    """
    for fn in nc.m.functions:
        for blk in fn.blocks:
            new_insts = []
            for inst in blk.instructions:
                si = inst.sync_info
                tname = type(inst).__name__
                if (
                    si is not None
                    and si.on_wait
                    and len(si.on_wait) > 1
                    and tname != "InstEventSemaphore"
                ):
                    for i, w in enumerate(si.on_wait):
                        new_insts.append(
                            mybir.InstEventSemaphore(
                                name=f"{inst.name}_hoistw{i}",
                                engine=inst.engine,
                                ins=[],
                                outs=[],
                                sync_info=mybir.SyncInfo(on_wait=[w], on_update=[]),
                            )
                        )
                    inst.sync_info = mybir.SyncInfo(
                        on_wait=[], on_update=si.on_update
                    )
                new_insts.append(inst)
            blk.instructions = new_insts


_NC_CACHE = {}


def _get_nc():
    if "nc" not in _NC_CACHE:
        _NC_CACHE["nc"] = build_bass()
    return _NC_CACHE["nc"]


def make_in_maps(inputs) -> list[dict]:
    o = np.asarray(inputs["output_rec"], dtype=np.float32)
    t = np.asarray(inputs["target_rec"], dtype=np.float32)
    mean = np.asarray(inputs["mean"], dtype=np.float32)
    log_var = np.asarray(inputs["log_var"], dtype=np.float32)
    oclas = np.asarray(inputs["output_clas"], dtype=np.float32)
    tclas = np.asarray(inputs["target_clas"]).astype(np.int64)

    # Only the real channel contributes to the inverse SSQ-STFT. Quantize
    # to fp8e4 (measured ~9e-4 rel err on the loss; tolerance is 2e-2).
    o_q = o[:, 0].astype(NP_FP8)  # [B, F, T]
    t_q = t[:, 0].astype(NP_FP8)

    onehot = np.zeros((B, C), dtype=np.float32)
    onehot[np.arange(B), tclas] = 1.0

    in_maps = []
    for c in range(N_CORES):
        s = slice(c * BS, (c + 1) * BS)
        # [BS, F, T] x2 -> [F, BS, {o,t}, T] -> f32 view [F, 8192]
        ot = np.empty((F, BS, 2, T), dtype=NP_FP8)
        ot[:, :, 0, :] = o_q[s].transpose(1, 0, 2)
        ot[:, :, 1, :] = t_q[s].transpose(1, 0, 2)
        sm = np.zeros((BS, SM_W), dtype=np.float32)
        sm[:, SM_MEAN:SM_MEAN + Z] = mean[s]
        sm[:, SM_LV:SM_LV + Z] = log_var[s]
        sm[:, SM_OC:SM_OC + C] = oclas[s]
        sm[:, SM_OH:SM_OH + C] = onehot[s]
        in_maps.append(
            {"ot_rec": ot.reshape(F, WCOL).view(np.float32), "smalls": sm}
        )
    return in_maps


def combine(results, weight) -> np.ndarray:
    """Host psum of the per-shard partial sums + loss-weight application."""
    w = np.asarray(weight, dtype=np.float64)
    total = 0.0
    for r in results:
        p = np.asarray(r["out"], dtype=np.float64)  # [8, 6]
        sq = p[:, 0:4].sum()
        kld = p[:, 4].sum()
        ce = p[:, 5].sum()
        total += (4.0 * w[0] * sq
                  - 0.5 * w[1] * (kld + BS * Z)
                  + (w[2] / B) * ce)
    return np.float32(total)


def kernel(**inputs) -> np.ndarray:
    in_maps = make_in_maps(inputs)
    nc = _get_nc()
    res = run_bass_kernel_spmd(nc, in_maps, list(range(N_CORES)))
    return combine(res.results, inputs["weight"])


# revision 14
# speedup vs baseline: 1.1816x; 1.1816x over previous
"""Trainium2 Bass kernel for nn_Couple_loss_62380105007762.

Loss = w0 * MSE + w1 * KLD + w2 * CE where
  sig(x)  = 2 * x[:, 0].sum(axis=F)                      (inverse SSQ-STFT, real channel only)
  MSE     = sum((sig(output_rec) - sig(target_rec))**2)
  KLD     = -0.5 * sum(1 + log_var - mean**2 - exp(log_var))
  CE      = mean cross-entropy(output_clas, target_clas)

Sharding: data-parallel over the batch dim (64 rows -> 8 cores x 8 rows).
Each core emits per-shard partial sums [8, 6] (4 sq quarters, kld, ce rows);
host psums the shards and applies the 3 loss weights.

v7 (v1 72.0us, v2 38.6us, v3 31.7us, v5 28.9us):
  - fp8e4 rec data (4x traffic; ~9e-4 rel err), interleaved [F, (b, {o,t}, T)].
  - DMA under an f32 VIEW of the fp8 tiles in 2-batch-row 1 MB chunks:
    HWDGE queue rate scales with descriptor size (descriptors split at 2048
    elements; 8 KB f32 descriptors measured 215 GB/s/queue vs ~160 for fp8).
  - DMA triggers + constant memsets hoisted into `main` right after the
    runtime-preamble InstCall: they depend on nothing in-run, and runs are
    serialized by the runtime, so they fire ~1.4us before the framework
    prologue barrier completes.
  - DoubleRow fp8 matmuls (32): one matmul = sum_f(o) - sum_f(t) per out
    column via the o|t interleave (contraction over 2x128 virtual rows).
  - psum split into FOUR [8, 1024] group tiles (2 banks each): group =
    (batch half, T half). Tile tracks dependencies per tile, so each
    group's square+accumulate fires as soon as ITS last matmul stops --
    only the final [8, 1024] square (~1.0us) sits on the tail instead of
    a [8, 2048] one (~2.0us).
  - PE warmup matmuls bridge the PE prologue -> first-chunk gap so HAM
    is at K=8/8 (2.4 GHz) when data lands.
  - smalls (mean/logvar/clas) ride ONE packed [8, 522] f32 DMA on the
    otherwise-idle gpsimd SWDGE ring.
"""

import numpy as np
import ml_dtypes
from contextlib import ExitStack

import concourse.bass as bass
import concourse.tile as tile
from concourse import mybir
from concourse.bass_utils import run_bass_kernel_spmd

N_CORES = 8
B, Z, F, T, C = 64, 256, 128, 2048, 5
BS = B // N_CORES   # batch rows per core
HB = BS // 2        # rows per batch half
WCOL = BS * 2 * T   # interleaved free dim: 32768 fp8 columns
WCOL32 = WCOL // 4  # same bytes as f32 columns
N_CHUNK = 512       # matmul output free dim (PSUM bank limit in fp32)
KQ = T // N_CHUNK   # 4 output slices per b
N_WARM = 10         # dummy matmuls bridging PE prologue -> first 1MB chunk

FP8 = mybir.dt.float8e4
NP_FP8 = ml_dtypes.float8_e4m3
FP32 = mybir.dt.float32
AX = mybir.AxisListType
ALU = mybir.AluOpType
ACTF = mybir.ActivationFunctionType
DR = mybir.MatmulPerfMode.DoubleRow
ET = mybir.EngineType

# packed smalls layout: [BS, SM_W] f32
SM_MEAN = 0               # cols [0, 256)    mean
SM_LV = Z                 # cols [256, 512)  log_var
SM_OC = 2 * Z             # cols [512, 517)  output_clas
SM_OH = 2 * Z + C         # cols [517, 522)  one-hot(target_clas)
SM_W = 2 * Z + 2 * C

# out columns: [sq_q0..sq_q3, kld, ce]
NO = 6


def build_bass(legalize: bool = True):
    nc = bass.Bass()

    ot_rec = nc.declare_dram_parameter("ot_rec", [F, WCOL32], FP32, isOutput=False)
    smalls = nc.declare_dram_parameter("smalls", [BS, SM_W], FP32, isOutput=False)
    out = nc.declare_dram_parameter("out", [BS, NO], FP32, isOutput=True)

    hoist_hwdge = []   # instruction names to move right after the main InstCall
    hoist_pool = []    # ... and after the Pool preamble memsets

    with tile.TileContext(nc) as tc:
        with ExitStack() as ctx:
            const_pool = ctx.enter_context(tc.tile_pool(name="const", bufs=1))
            d_pool = ctx.enter_context(tc.tile_pool(name="dpool", bufs=BS // 2))
            ps_pool = ctx.enter_context(tc.tile_pool(name="ps", bufs=1, space="PSUM"))
            small = ctx.enter_context(tc.tile_pool(name="small", bufs=1))

            # ---- big data chunks; DMA issued under an f32 view ----
            QUEUES = {0: nc.sync, 1: nc.scalar, 2: nc.sync, 3: nc.scalar}
            sm_t = small.tile([BS, SM_W], FP32, tag="sm")
            # NOT hoisted: the Pool preamble Drain waits for in-flight
            # SWDGE DMAs, so a pre-barrier SWDGE trigger stalls the global
            # prologue barrier until the transfer lands (~5us, v6).
            nc.gpsimd.dma_start(sm_t[:], smalls[:, :])
            # 2-b chunks: [128, 8192] fp8 = 1 MB, 8 KB f32-view descriptors
            pairs = []
            for p in range(BS // 2):
                ch = d_pool.tile([F, 4 * T], FP8, tag="d")
                sl32 = slice(p * T, (p + 1) * T)
                i_d = QUEUES[p].dma_start(ch[:].bitcast(FP32), ot_rec[:, sl32])
                hoist_hwdge.append(i_d.ins.name)
                pairs.append(ch)
            chunks = []
            for b in range(BS):
                off = (b % 2) * 2 * T
                chunks.append(pairs[b // 2][:, off:off + 2 * T])

            # ---- constants (no DMA): selector weights + warmup junk ----
            # W[:, 8] = +1, W[:, 24] = -1, rest 0.  DoubleRow stationary for
            # batch row b: W viewed as [128, j:2(x16), m:8] at offset 8-b
            # => (j=0, m=b) hits col 8 (+1), (j=1, m=b) hits col 24 (-1).
            w_sel = const_pool.tile([F, 32], FP8, tag="wsel")
            i1 = nc.vector.memset(w_sel[:], 0.0)
            i2 = nc.vector.memset(w_sel[:, 8:9], 1.0)
            i3 = nc.vector.memset(w_sel[:, 24:25], -1.0)
            warm_in = const_pool.tile([F, N_CHUNK], FP8, tag="warmin")
            i4 = nc.vector.memset(warm_in[:], 0.0)
            hoist_hwdge += [i1.ins.name, i2.ins.name, i3.ins.name, i4.ins.name]

            # psum: four [8, 1024] group tiles (2 banks each) = all 8 banks.
            # group q = (batch half, T half); per-group squares can then
            # fire on their own group's last matmul (Tile deps are
            # tile-granular).
            ps_q0 = ps_pool.tile([BS, 2 * N_CHUNK], FP32, tag="q0")
            ps_q1 = ps_pool.tile([BS, 2 * N_CHUNK], FP32, tag="q1")
            ps_q2 = ps_pool.tile([BS, 2 * N_CHUNK], FP32, tag="q2")
            ps_q3 = ps_pool.tile([BS, 2 * N_CHUNK], FP32, tag="q3")
            ps_q = [ps_q0, ps_q1, ps_q2, ps_q3]
            # out tile: [8, 6] = sq_q0..q3 | kld | ce
            sums = small.tile([BS, NO], FP32, tag="sums")

            # ---- PE warmup: HAM unthrottles after ~3.4us of activity.
            # Writes [1, 512] garbage into q3; the first real q3 matmul
            # opens its accumulation group with start=True, clearing it.
            for i in range(N_WARM):
                nc.tensor.matmul(ps_q3[0:1, 0:N_CHUNK], w_sel[:, 0:1],
                                 warm_in[:], start=True, stop=True,
                                 skip_group_check=True)

            # ---- KLD / CE on the packed smalls tile (off critical path) ----
            m_t = sm_t[:, SM_MEAN:SM_MEAN + Z]
            lv_t = sm_t[:, SM_LV:SM_LV + Z]
            oc_t = sm_t[:, SM_OC:SM_OC + C]
            oh_t = sm_t[:, SM_OH:SM_OH + C]

            msq_sum = small.tile([BS, 1], FP32, tag="msq")
            e_sum = small.tile([BS, 1], FP32, tag="esum")
            lv_sum = small.tile([BS, 1], FP32, tag="lvsum")
            kl_junk = small.tile([BS, Z], FP32, tag="klj")
            kl_junk2 = small.tile([BS, Z], FP32, tag="klj2")
            kl_tmp = small.tile([BS, 1], FP32, tag="kltmp")
            nc.vector.tensor_tensor(kl_junk[:], m_t, m_t, ALU.mult)
            nc.vector.reduce_sum(msq_sum[:], kl_junk[:], axis=AX.X)
            nc.scalar.activation(kl_junk2[:], lv_t, ACTF.Exp, accum_out=e_sum[:])
            nc.vector.reduce_sum(lv_sum[:], lv_t, axis=AX.X)
            nc.vector.tensor_tensor(kl_tmp[:], lv_sum[:], msq_sum[:], ALU.subtract)
            nc.vector.tensor_tensor(sums[:, 4:5], kl_tmp[:], e_sum[:], ALU.subtract)

            # CE rows: ce_row = rowmax + log(sum(exp(oc - rowmax))) - oc[b, y_b]
            rowmax = small.tile([BS, 1], FP32, tag="rmax")
            nmax = small.tile([BS, 1], FP32, tag="nmax")
            sumexp = small.tile([BS, 1], FP32, tag="sexp")
            lse = small.tile([BS, 1], FP32, tag="lse")
            picked = small.tile([BS, 1], FP32, tag="picked")
            ce_junk = small.tile([BS, C], FP32, tag="cej")
            ce_junk2 = small.tile([BS, C], FP32, tag="cej2")
            ce_tmp = small.tile([BS, 1], FP32, tag="cetmp")
            nc.vector.reduce_max(rowmax[:], oc_t, axis=AX.X)
            nc.vector.tensor_scalar_mul(nmax[:], rowmax[:], -1.0)
            nc.scalar.activation(
                ce_junk[:], oc_t, ACTF.Exp, bias=nmax[:], accum_out=sumexp[:]
            )
            nc.scalar.activation(lse[:], sumexp[:], ACTF.Ln)
            nc.vector.tensor_tensor(ce_junk2[:], oc_t, oh_t, ALU.mult)
            nc.vector.reduce_sum(picked[:], ce_junk2[:], axis=AX.X)
            nc.vector.tensor_tensor(ce_tmp[:], rowmax[:], lse[:], ALU.add)
            nc.vector.tensor_tensor(sums[:, 5:6], ce_tmp[:], picked[:],
                                    ALU.subtract)

            # ---- main MSE stream ----
            # DoubleRow: out[m,n] = sum_f W3[f,0,m]*x[f,0,n] + W3[f,1,m]*x[f,1,n]
            #          = sum_f o[b, f, n] - sum_f t[b, f, n]  for row m = b
            # (b0-3 land in rows 0-3 of the q0/q1 groups; b4-7 in rows 4-7
            # of q2/q3; unused rows stay 0 and add nothing)
            w3 = w_sel[:].rearrange("p (j m) -> p j m", j=2)  # [128, 2, 16]
            for b in range(BS):
                wb = w3[:, :, 8 - b:16 - b]                    # [128, 2, 8]
                c3 = chunks[b].rearrange("p (j n) -> p j n", j=2)
                for k in range(KQ):
                    q = ps_q[(b // HB) * 2 + (k // 2)]
                    col = (k % 2) * N_CHUNK
                    nc.tensor.matmul(
                        q[:, col:col + N_CHUNK],
                        wb, c3[:, :, k * N_CHUNK:(k + 1) * N_CHUNK],
                        start=(b % HB == 0),
                        stop=(b % HB == HB - 1),
                        perf_mode=DR,
                    )
                if b % HB == HB - 1:
                    for kh in range(2):
                        qi = (b // HB) * 2 + kh
                        sq_junk = small.tile([BS, 2 * N_CHUNK], FP32,
                                             tag=f"sqj{qi}")
                        nc.scalar.activation(
                            sq_junk[:], ps_q[qi][:], ACTF.Square,
                            accum_out=sums[:, qi:qi + 1],
                        )

            nc.sync.dma_start(out[:, :], sums[:])

    if legalize:
        _legalize_multi_waits(nc)
    _hoist_preamble(nc, hoist_hwdge, hoist_pool)
    mybir.codegen_inst_isa_subclasses(nc)
    return nc


def _hoist_preamble(nc, names_after_call, names_pool):
    """Move dependency-free DMA triggers / memsets from the body block into
    `main`, ahead of the framework prologue barrier: HWDGE triggers + DVE
    memsets right after the runtime-preamble InstCall, SWDGE (Pool)
    triggers after the Pool preamble memsets (DGE ring init). Safe because
    NEFF executions are serialized by the runtime and these instructions
    depend on nothing produced in-run."""
    fn = nc.m.functions[0]
    main = fn.blocks[0]
    assert main.name == "main"
    wanted = set(names_after_call) | set(names_pool)
    moved = {}
    for blk in fn.blocks[1:]:
        keep = []
        for inst in blk.instructions:
            if inst.name in wanted:
                moved[inst.name] = inst
            else:
                keep.append(inst)
        blk.instructions = keep

    new_main = []
    for inst in main.instructions:
        new_main.append(inst)
        if type(inst).__name__ == "InstCall":
            for n in names_after_call:
                if n in moved:
                    new_main.append(moved[n])
    final = []
    pool_done = False
    for inst in new_main:
        if (not pool_done and type(inst).__name__ == "InstDrain"
                and inst.engine == ET.Pool):
            for n in names_pool:
                if n in moved:
                    final.append(moved[n])
            pool_done = True
        final.append(inst)
    main.instructions = final


def _legalize_multi_waits(nc):
    """walrus rejects TPB compute instructions carrying more than one sync
    wait. Hoist extra waits onto standalone InstEventSemaphore instructions
    on the same engine. DMA instructions keep their waits (DGE path).
    """
    for fn in nc.m.functions:
        for blk in fn.blocks:
            new_insts = []
            for inst in blk.instructions:
                si = inst.sync_info
                tname = type(inst).__name__
                if (
                    si is not None
                    and si.on_wait
                    and len(si.on_wait) > 1
                    and tname != "InstEventSemaphore"
                ):
                    for i, w in enumerate(si.on_wait):
                        new_insts.append(
                            mybir.InstEventSemaphore(
                                name=f"{inst.name}_hoistw{i}",
                                engine=inst.engine,
                                ins=[],
                                outs=[],
                                sync_info=mybir.SyncInfo(on_wait=[w], on_update=[]),
                            )
                        )
                    inst.sync_info = mybir.SyncInfo(
                        on_wait=[], on_update=si.on_update
                    )
                new_insts.append(inst)
            blk.instructions = new_insts


_NC_CACHE = {}


def _get_nc():
    if "nc" not in _NC_CACHE:
        _NC_CACHE["nc"] = build_bass()
    return _NC_CACHE["nc"]


def make_in_maps(inputs) -> list[dict]:
    o = np.asarray(inputs["output_rec"], dtype=np.float32)
    t = np.asarray(inputs["target_rec"], dtype=np.float32)
    mean = np.asarray(inputs["mean"], dtype=np.float32)
    log_var = np.asarray(inputs["log_var"], dtype=np.float32)
    oclas = np.asarray(inputs["output_clas"], dtype=np.float32)
    tclas = np.asarray(inputs["target_clas"]).astype(np.int64)

    # Only the real channel contributes to the inverse SSQ-STFT. Quantize
    # to fp8e4 (measured ~9e-4 rel err on the loss; tolerance is 2e-2).
    o_q = o[:, 0].astype(NP_FP8)  # [B, F, T]
    t_q = t[:, 0].astype(NP_FP8)

    onehot = np.zeros((B, C), dtype=np.float32)
    onehot[np.arange(B), tclas] = 1.0

    in_maps = []
    for c in range(N_CORES):
        s = slice(c * BS, (c + 1) * BS)
        # [BS, F, T] x2 -> [F, BS, {o,t}, T] -> f32 view [F, 8192]
        ot = np.empty((F, BS, 2, T), dtype=NP_FP8)
        ot[:, :, 0, :] = o_q[s].transpose(1, 0, 2)
        ot[:, :, 1, :] = t_q[s].transpose(1, 0, 2)
        sm = np.zeros((BS, SM_W), dtype=np.float32)
        sm[:, SM_MEAN:SM_MEAN + Z] = mean[s]
        sm[:, SM_LV:SM_LV + Z] = log_var[s]
        sm[:, SM_OC:SM_OC + C] = oclas[s]
        sm[:, SM_OH:SM_OH + C] = onehot[s]
        in_maps.append(
            {"ot_rec": ot.reshape(F, WCOL).view(np.float32), "smalls": sm}
        )
    return in_maps


def combine(results, weight) -> np.ndarray:
    """Host psum of the per-shard partial sums + loss-weight application."""
    w = np.asarray(weight, dtype=np.float64)
    total = 0.0
    for r in results:
        p = np.asarray(r["out"], dtype=np.float64)  # [8, 6]
        sq = p[:, 0:4].sum()
        kld = p[:, 4].sum()
        ce = p[:, 5].sum()
        total += (4.0 * w[0] * sq
                  - 0.5 * w[1] * (kld + BS * Z)
                  + (w[2] / B) * ce)
    return np.float32(total)


def kernel(**inputs) -> np.ndarray:
    in_maps = make_in_maps(inputs)
    nc = _get_nc()
    res = run_bass_kernel_spmd(nc, in_maps, list(range(N_CORES)))
    return combine(res.results, inputs["weight"])


# revision 15
# speedup vs baseline: 1.2779x; 1.0815x over previous
"""Trainium2 Bass kernel for nn_Couple_loss_62380105007762.

Loss = w0 * MSE + w1 * KLD + w2 * CE where
  sig(x)  = 2 * x[:, 0].sum(axis=F)                      (inverse SSQ-STFT, real channel only)
  MSE     = sum((sig(output_rec) - sig(target_rec))**2)
  KLD     = -0.5 * sum(1 + log_var - mean**2 - exp(log_var))
  CE      = mean cross-entropy(output_clas, target_clas)

Sharding: data-parallel over the batch dim (64 rows -> 8 cores x 8 rows).
Each core emits per-shard partial sums [8, 6] (4 sq quarters, kld, ce rows);
host psums the shards and applies the 3 loss weights.

v8 (v1 72.0us, v2 38.6us, v3 31.7us, v5 28.9us):
  - fp8e4 rec data (4x traffic; ~9e-4 rel err), interleaved [F, (b, {o,t}, T)].
  - DMA under an f32 VIEW of the fp8 tiles in 2-batch-row 1 MB chunks:
    HWDGE queue rate scales with descriptor size (descriptors split at 2048
    elements; 8 KB f32 descriptors measured 215 GB/s/queue vs ~160 for fp8).
  - DMA triggers + constant memsets hoisted into `main` right after the
    runtime-preamble InstCall: they depend on nothing in-run, and runs are
    serialized by the runtime, so they fire ~1.4us before the framework
    prologue barrier completes.
  - DoubleRow fp8 matmuls (32): one matmul = sum_f(o) - sum_f(t) per out
    column via the o|t interleave (contraction over 2x128 virtual rows).
  - psum split into FOUR [8, 1024] group tiles (2 banks each): group =
    (batch half, T half). Tile tracks dependencies per tile, so each
    group's square+accumulate fires as soon as ITS last matmul stops --
    only the final [8, 1024] square (~1.0us) sits on the tail instead of
    a [8, 2048] one (~2.0us).
  - PE warmup matmuls bridge the PE prologue -> first-chunk gap so HAM
    is at K=8/8 (2.4 GHz) when data lands.
  - smalls (mean/logvar/clas) ride ONE packed [8, 522] f32 DMA on the
    otherwise-idle gpsimd SWDGE ring.
"""

import numpy as np
import ml_dtypes
from contextlib import ExitStack

import concourse.bass as bass
import concourse.tile as tile
from concourse import mybir
from concourse.bass_utils import run_bass_kernel_spmd

N_CORES = 8
B, Z, F, T, C = 64, 256, 128, 2048, 5
BS = B // N_CORES   # batch rows per core
HB = BS // 2        # rows per batch half
WCOL = BS * 2 * T   # interleaved free dim: 32768 fp8 columns
WCOL32 = WCOL // 4  # same bytes as f32 columns
N_CHUNK = 512       # matmul output free dim (PSUM bank limit in fp32)
KQ = T // N_CHUNK   # 4 output slices per b
N_WARM = 10         # dummy matmuls bridging PE prologue -> first 1MB chunk

FP8 = mybir.dt.float8e4
NP_FP8 = ml_dtypes.float8_e4m3
FP32 = mybir.dt.float32
AX = mybir.AxisListType
ALU = mybir.AluOpType
ACTF = mybir.ActivationFunctionType
DR = mybir.MatmulPerfMode.DoubleRow
ET = mybir.EngineType

# packed smalls layout: [BS, SM_W] f32
SM_MEAN = 0               # cols [0, 256)    mean
SM_LV = Z                 # cols [256, 512)  log_var
SM_OC = 2 * Z             # cols [512, 517)  output_clas
SM_OH = 2 * Z + C         # cols [517, 522)  one-hot(target_clas)
SM_W = 2 * Z + 2 * C

# out columns: [sq_q0..sq_q3, kld, ce]
NO = 6


def build_bass(legalize: bool = True):
    nc = bass.Bass()

    ot_rec = nc.declare_dram_parameter("ot_rec", [F, WCOL32], FP32, isOutput=False)
    smalls = nc.declare_dram_parameter("smalls", [BS, SM_W], FP32, isOutput=False)
    out = nc.declare_dram_parameter("out", [BS, NO], FP32, isOutput=True)

    hoist_hwdge = []   # instruction names to move right after the main InstCall
    hoist_pool = []    # ... and after the Pool preamble memsets

    with tile.TileContext(nc) as tc:
        with ExitStack() as ctx:
            const_pool = ctx.enter_context(tc.tile_pool(name="const", bufs=1))
            d_pool = ctx.enter_context(tc.tile_pool(name="dpool", bufs=BS // 2))
            ps_pool = ctx.enter_context(tc.tile_pool(name="ps", bufs=1, space="PSUM"))
            small = ctx.enter_context(tc.tile_pool(name="small", bufs=1))

            # ---- big data chunks; DMA issued under an f32 view ----
            sm_t = small.tile([BS, SM_W], FP32, tag="sm")
            # NOT hoisted: the Pool preamble Drain waits for in-flight
            # SWDGE DMAs, so a pre-barrier SWDGE trigger stalls the global
            # prologue barrier until the transfer lands (~5us, v6).
            nc.gpsimd.dma_start(sm_t[:], smalls[:, :])
            # 2-b chunks: [128, 8192] fp8 = 1 MB, 8 KB f32-view descriptors.
            # The two HWDGE rings start skewed: whichever ring's first
            # descriptors reach the SDMA engines first streams ~solo for
            # ~3us before the other ramps. Serialize trigger order with a
            # semaphore so sync (carrying B01, needed first) always wins.
            ring_sem = nc.alloc_semaphore("ring_order")
            pairs = [None] * (BS // 2)
            for p in (0, 2):
                ch = d_pool.tile([F, 4 * T], FP8, tag="d")
                sl32 = slice(p * T, (p + 1) * T)
                i_d = nc.sync.dma_start(ch[:].bitcast(FP32), ot_rec[:, sl32])
                hoist_hwdge.append(i_d.ins.name)
                pairs[p] = ch
            i_inc = nc.sync.sem_inc(ring_sem, 1)
            hoist_hwdge.append(i_inc.ins.name)
            i_w = nc.scalar.wait_ge(ring_sem, 1)
            hoist_hwdge.append(i_w.ins.name)
            for p in (1, 3):
                ch = d_pool.tile([F, 4 * T], FP8, tag="d")
                sl32 = slice(p * T, (p + 1) * T)
                i_d = nc.scalar.dma_start(ch[:].bitcast(FP32), ot_rec[:, sl32])
                hoist_hwdge.append(i_d.ins.name)
                pairs[p] = ch
            chunks = []
            for b in range(BS):
                off = (b % 2) * 2 * T
                chunks.append(pairs[b // 2][:, off:off + 2 * T])

            # ---- constants (no DMA): selector weights + warmup junk ----
            # W[:, 8] = +1, W[:, 24] = -1, rest 0.  DoubleRow stationary for
            # batch row b: W viewed as [128, j:2(x16), m:8] at offset 8-b
            # => (j=0, m=b) hits col 8 (+1), (j=1, m=b) hits col 24 (-1).
            w_sel = const_pool.tile([F, 32], FP8, tag="wsel")
            i1 = nc.vector.memset(w_sel[:], 0.0)
            i2 = nc.vector.memset(w_sel[:, 8:9], 1.0)
            i3 = nc.vector.memset(w_sel[:, 24:25], -1.0)
            warm_in = const_pool.tile([F, N_CHUNK], FP8, tag="warmin")
            i4 = nc.vector.memset(warm_in[:], 0.0)
            hoist_hwdge += [i1.ins.name, i2.ins.name, i3.ins.name, i4.ins.name]

            # psum: four [8, 1024] group tiles (2 banks each) = all 8 banks.
            # group q = (batch half, T half); per-group squares can then
            # fire on their own group's last matmul (Tile deps are
            # tile-granular).
            ps_q0 = ps_pool.tile([BS, 2 * N_CHUNK], FP32, tag="q0")
            ps_q1 = ps_pool.tile([BS, 2 * N_CHUNK], FP32, tag="q1")
            ps_q2 = ps_pool.tile([BS, 2 * N_CHUNK], FP32, tag="q2")
            ps_q3 = ps_pool.tile([BS, 2 * N_CHUNK], FP32, tag="q3")
            ps_q = [ps_q0, ps_q1, ps_q2, ps_q3]
            # out tile: [8, 6] = sq_q0..q3 | kld | ce
            sums = small.tile([BS, NO], FP32, tag="sums")

            # ---- PE warmup: HAM unthrottles after ~3.4us of activity.
            # Writes [1, 512] garbage into q3; the first real q3 matmul
            # opens its accumulation group with start=True, clearing it.
            for i in range(N_WARM):
                nc.tensor.matmul(ps_q3[0:1, 0:N_CHUNK], w_sel[:, 0:1],
                                 warm_in[:], start=True, stop=True,
                                 skip_group_check=True)

            # ---- KLD / CE on the packed smalls tile (off critical path) ----
            m_t = sm_t[:, SM_MEAN:SM_MEAN + Z]
            lv_t = sm_t[:, SM_LV:SM_LV + Z]
            oc_t = sm_t[:, SM_OC:SM_OC + C]
            oh_t = sm_t[:, SM_OH:SM_OH + C]

            msq_sum = small.tile([BS, 1], FP32, tag="msq")
            e_sum = small.tile([BS, 1], FP32, tag="esum")
            lv_sum = small.tile([BS, 1], FP32, tag="lvsum")
            kl_junk = small.tile([BS, Z], FP32, tag="klj")
            kl_junk2 = small.tile([BS, Z], FP32, tag="klj2")
            kl_tmp = small.tile([BS, 1], FP32, tag="kltmp")
            nc.vector.tensor_tensor(kl_junk[:], m_t, m_t, ALU.mult)
            nc.vector.reduce_sum(msq_sum[:], kl_junk[:], axis=AX.X)
            nc.scalar.activation(kl_junk2[:], lv_t, ACTF.Exp, accum_out=e_sum[:])
            nc.vector.reduce_sum(lv_sum[:], lv_t, axis=AX.X)
            nc.vector.tensor_tensor(kl_tmp[:], lv_sum[:], msq_sum[:], ALU.subtract)
            nc.vector.tensor_tensor(sums[:, 4:5], kl_tmp[:], e_sum[:], ALU.subtract)

            # CE rows: ce_row = rowmax + log(sum(exp(oc - rowmax))) - oc[b, y_b]
            rowmax = small.tile([BS, 1], FP32, tag="rmax")
            nmax = small.tile([BS, 1], FP32, tag="nmax")
            sumexp = small.tile([BS, 1], FP32, tag="sexp")
            lse = small.tile([BS, 1], FP32, tag="lse")
            picked = small.tile([BS, 1], FP32, tag="picked")
            ce_junk = small.tile([BS, C], FP32, tag="cej")
            ce_junk2 = small.tile([BS, C], FP32, tag="cej2")
            ce_tmp = small.tile([BS, 1], FP32, tag="cetmp")
            nc.vector.reduce_max(rowmax[:], oc_t, axis=AX.X)
            nc.vector.tensor_scalar_mul(nmax[:], rowmax[:], -1.0)
            nc.scalar.activation(
                ce_junk[:], oc_t, ACTF.Exp, bias=nmax[:], accum_out=sumexp[:]
            )
            nc.scalar.activation(lse[:], sumexp[:], ACTF.Ln)
            nc.vector.tensor_tensor(ce_junk2[:], oc_t, oh_t, ALU.mult)
            nc.vector.reduce_sum(picked[:], ce_junk2[:], axis=AX.X)
            nc.vector.tensor_tensor(ce_tmp[:], rowmax[:], lse[:], ALU.add)
            nc.vector.tensor_tensor(sums[:, 5:6], ce_tmp[:], picked[:],
                                    ALU.subtract)

            # ---- main MSE stream ----
            # DoubleRow: out[m,n] = sum_f W3[f,0,m]*x[f,0,n] + W3[f,1,m]*x[f,1,n]
            #          = sum_f o[b, f, n] - sum_f t[b, f, n]  for row m = b
            # (b0-3 land in rows 0-3 of the q0/q1 groups; b4-7 in rows 4-7
            # of q2/q3; unused rows stay 0 and add nothing)
            w3 = w_sel[:].rearrange("p (j m) -> p j m", j=2)  # [128, 2, 16]
            for b in range(BS):
                wb = w3[:, :, 8 - b:16 - b]                    # [128, 2, 8]
                c3 = chunks[b].rearrange("p (j n) -> p j n", j=2)
                for k in range(KQ):
                    q = ps_q[(b // HB) * 2 + (k // 2)]
                    col = (k % 2) * N_CHUNK
                    nc.tensor.matmul(
                        q[:, col:col + N_CHUNK],
                        wb, c3[:, :, k * N_CHUNK:(k + 1) * N_CHUNK],
                        start=(b % HB == 0),
                        stop=(b % HB == HB - 1),
                        perf_mode=DR,
                    )
                if b % HB == HB - 1:
                    for kh in range(2):
                        qi = (b // HB) * 2 + kh
                        sq_junk = small.tile([BS, 2 * N_CHUNK], FP32,
                                             tag=f"sqj{qi}")
                        nc.scalar.activation(
                            sq_junk[:], ps_q[qi][:], ACTF.Square,
                            accum_out=sums[:, qi:qi + 1],
                        )

            nc.sync.dma_start(out[:, :], sums[:])

    if legalize:
        _legalize_multi_waits(nc)
    _hoist_preamble(nc, hoist_hwdge, hoist_pool)
    mybir.codegen_inst_isa_subclasses(nc)
    return nc


def _hoist_preamble(nc, names_after_call, names_pool):
    """Move dependency-free DMA triggers / memsets from the body block into
    `main`, ahead of the framework prologue barrier: HWDGE triggers + DVE
    memsets right after the runtime-preamble InstCall, SWDGE (Pool)
    triggers after the Pool preamble memsets (DGE ring init). Safe because
    NEFF executions are serialized by the runtime and these instructions
    depend on nothing produced in-run."""
    fn = nc.m.functions[0]
    main = fn.blocks[0]
    assert main.name == "main"
    wanted = set(names_after_call) | set(names_pool)
    moved = {}
    for blk in fn.blocks[1:]:
        keep = []
        for inst in blk.instructions:
            if inst.name in wanted:
                moved[inst.name] = inst
            else:
                keep.append(inst)
        blk.instructions = keep

    new_main = []
    for inst in main.instructions:
        new_main.append(inst)
        if type(inst).__name__ == "InstCall":
            for n in names_after_call:
                if n in moved:
                    new_main.append(moved[n])
    final = []
    pool_done = False
    for inst in new_main:
        if (not pool_done and type(inst).__name__ == "InstDrain"
                and inst.engine == ET.Pool):
            for n in names_pool:
                if n in moved:
                    final.append(moved[n])
            pool_done = True
        final.append(inst)
    main.instructions = final


def _legalize_multi_waits(nc):
    """walrus rejects TPB compute instructions carrying more than one sync
    wait. Hoist extra waits onto standalone InstEventSemaphore instructions
    on the same engine. DMA instructions keep their waits (DGE path).
    """
    for fn in nc.m.functions:
        for blk in fn.blocks:
            new_insts = []
            for inst in blk.instructions:
                si = inst.sync_info
                tname = type(inst).__name__
                if (
                    si is not None
                    and si.on_wait
                    and len(si.on_wait) > 1
                    and tname != "InstEventSemaphore"
                ):
                    for i, w in enumerate(si.on_wait):
                        new_insts.append(
                            mybir.InstEventSemaphore(
                                name=f"{inst.name}_hoistw{i}",
                                engine=inst.engine,
                                ins=[],
                                outs=[],
                                sync_info=mybir.SyncInfo(on_wait=[w], on_update=[]),
                            )
                        )
                    inst.sync_info = mybir.SyncInfo(
                        on_wait=[], on_update=si.on_update
                    )
                new_insts.append(inst)
            blk.instructions = new_insts


_NC_CACHE = {}


def _get_nc():
    if "nc" not in _NC_CACHE:
        _NC_CACHE["nc"] = build_bass()
    return _NC_CACHE["nc"]


def make_in_maps(inputs) -> list[dict]:
    o = np.asarray(inputs["output_rec"], dtype=np.float32)
    t = np.asarray(inputs["target_rec"], dtype=np.float32)
    mean = np.asarray(inputs["mean"], dtype=np.float32)
    log_var = np.asarray(inputs["log_var"], dtype=np.float32)
    oclas = np.asarray(inputs["output_clas"], dtype=np.float32)
    tclas = np.asarray(inputs["target_clas"]).astype(np.int64)

    # Only the real channel contributes to the inverse SSQ-STFT. Quantize
    # to fp8e4 (measured ~9e-4 rel err on the loss; tolerance is 2e-2).
    o_q = o[:, 0].astype(NP_FP8)  # [B, F, T]
    t_q = t[:, 0].astype(NP_FP8)

    onehot = np.zeros((B, C), dtype=np.float32)
    onehot[np.arange(B), tclas] = 1.0

    in_maps = []
    for c in range(N_CORES):
        s = slice(c * BS, (c + 1) * BS)
        # [BS, F, T] x2 -> [F, BS, {o,t}, T] -> f32 view [F, 8192]
        ot = np.empty((F, BS, 2, T), dtype=NP_FP8)
        ot[:, :, 0, :] = o_q[s].transpose(1, 0, 2)
        ot[:, :, 1, :] = t_q[s].transpose(1, 0, 2)
        sm = np.zeros((BS, SM_W), dtype=np.float32)
        sm[:, SM_MEAN:SM_MEAN + Z] = mean[s]
        sm[:, SM_LV:SM_LV + Z] = log_var[s]
        sm[:, SM_OC:SM_OC + C] = oclas[s]
        sm[:, SM_OH:SM_OH + C] = onehot[s]
        in_maps.append(
            {"ot_rec": ot.reshape(F, WCOL).view(np.float32), "smalls": sm}
        )
    return in_maps


def combine(results, weight) -> np.ndarray:
    """Host psum of the per-shard partial sums + loss-weight application."""
    w = np.asarray(weight, dtype=np.float64)
    total = 0.0
    for r in results:
        p = np.asarray(r["out"], dtype=np.float64)  # [8, 6]
        sq = p[:, 0:4].sum()
        kld = p[:, 4].sum()
        ce = p[:, 5].sum()
        total += (4.0 * w[0] * sq
                  - 0.5 * w[1] * (kld + BS * Z)
                  + (w[2] / B) * ce)
    return np.float32(total)


def kernel(**inputs) -> np.ndarray:
    in_maps = make_in_maps(inputs)
    nc = _get_nc()
    res = run_bass_kernel_spmd(nc, in_maps, list(range(N_CORES)))
    return combine(res.results, inputs["weight"])
